# revision 47
# baseline (speedup 1.0000x reference)
"""Causal multi-head attention (B=4, S=2048, D=1024, H=16, hd=64) on 8
Trainium2 NeuronCores.

Sharding: batch (4-way) x head-group (2-way). Core c handles batch c//2 and
heads [8*(c%2), 8*(c%2)+8). Each core computes its heads' contribution to the
output projection; the host sums the two partials per batch and adds bo.

Per-core device program:
  - Projections run in bf16 (x and Wq/Wk/Wv pre-cast on the host): the
    1024-deep contraction washes the quantization out, FWL doubles weight
    loads, and x DMA traffic halves. Q/K/V results are evicted to fp32r
    (K/Q) and bf16 (V) for the attention matmuls.
  - K/V/Q projections are emitted per 512-query window as six "quarters"
    (K/V/Q x two head-pairs, one [128,1024] PSUM tile each), interleaved
    into the PREVIOUS window's attention stream as PE filler so the PE
    never idles long enough for HAM to re-throttle the clock.
  - Attention per window in the transposed layout: scoresT = K_chunk @ Q^T
    (row-tiled K=64 matmuls, head parities at partitions 0-63/64-127 run
    concurrently), exp(s/8) split across engines: most chunks on ACT
    (spline exp), every third full-width chunk on DVE via the Schraudolph
    bit-trick (int16(A*s + B) reinterpreted as bf16; ~3% rel err, washes
    out after softmax normalization). Causal block-skipping plus a
    triangular bf16 mask on diagonal subblocks. attn-out accumulates via
    bf16 V_aug matmuls whose fused ones-column also yields the softmax
    denominator Z (PSUM row 64).
  - Normalization: 1/Z computed EXACTLY on DVE with zero ACT table
    pressure: 32x32 stream-transpose of the Z row block, reciprocal on the
    [32,32]-strided column view (8 cyc/elem on only 32 elems/lane instead
    of 1024), transpose back, gpsimd partition_broadcast, multiply.
  - Output projection of window w-1 interleaved into window w (more PE
    filler); eviction via ACT copy; partial [S, D] DMA'd out.
"""
import numpy as np
import ml_dtypes

import concourse.mybir as mybir
from concourse import bacc
from concourse.tile import TileContext
from concourse.bass_utils import run_bass_kernel_spmd

FP32 = mybir.dt.float32
FP32R = mybir.dt.float32r
BF16 = mybir.dt.bfloat16
INT16 = mybir.dt.int16
EXP = mybir.ActivationFunctionType.Exp
IDENT = mybir.ActivationFunctionType.Identity

B, S, D = 4, 2048, 1024
H, HD = 16, 64
NCORES = 8
HPG = 8              # heads per group (per core)
GD = HPG * HD        # 512: group head-dim width
W = 512              # query window
NW = S // W          # 4
KCH = 128            # key chunk
NKC = S // KCH       # 16
DC = 128             # D contraction chunk
NDC = D // DC        # 8
SCALE = 1.0 / 8.0    # 1/sqrt(hd)

# Schraudolph fast-exp, bf16 flavor: exp(y) ~= bits(int16(A*y + B)) as bf16.
# C=5.5 calibrated for min max-rel-err (~3.3%) over y in [-8, 4].
FEXP_A = 128.0 / float(np.log(2.0))
FEXP_B = 127.0 * 128.0 - 5.5

_CACHE = {}


def _build_program():
    nc = bacc.Bacc("TRN2", target_bir_lowering=False, debug=False,
                   num_devices=NCORES)

    xT = nc.dram_tensor("xT", [D, S], BF16, kind="ExternalInput").ap()
    wq = nc.dram_tensor("wq", [D, GD], BF16, kind="ExternalInput").ap()
    wk = nc.dram_tensor("wk", [D, GD], BF16, kind="ExternalInput").ap()
    wv = nc.dram_tensor("wv", [D, GD], BF16, kind="ExternalInput").ap()
    wo = nc.dram_tensor("wo", [GD, D], FP32, kind="ExternalInput").ap()
    bq2 = nc.dram_tensor("bq2", [128, 4], FP32, kind="ExternalInput").ap()
    bk2 = nc.dram_tensor("bk2", [128, 4], FP32, kind="ExternalInput").ap()
    out = nc.dram_tensor("out", [S, D], FP32, kind="ExternalOutput").ap()

    with TileContext(nc) as tc:
        with (
            tc.tile_pool(name="xt", bufs=10) as xt_pool,
            tc.tile_pool(name="wb", bufs=24) as wb_pool,
            tc.tile_pool(name="wo", bufs=8) as wo_pool,
            tc.tile_pool(name="kt", bufs=4) as kt_pool,
            tc.tile_pool(name="vst", bufs=16) as v_pool,
            tc.tile_pool(name="qt", bufs=8) as qt_pool,
            tc.tile_pool(name="et", bufs=8) as exp_pool,
            tc.tile_pool(name="ao", bufs=8) as ao_pool,
            tc.tile_pool(name="zz", bufs=2) as zz_pool,
            tc.tile_pool(name="zn", bufs=1) as zn_pool,
            tc.tile_pool(name="cst", bufs=1) as cst_pool,
            tc.tile_pool(name="ob", bufs=3) as out_pool,
            tc.tile_pool(name="ps", bufs=4, space="PSUM") as ps2,
        ):
            # ---- constants: biases, triangular mask ----
            bq_t = cst_pool.tile([128, 4], FP32, tag="bq")
            bk_t = cst_pool.tile([128, 4], FP32, tag="bk")
            nc.sync.dma_start(out=bq_t[:], in_=bq2[:])
            nc.sync.dma_start(out=bk_t[:], in_=bk2[:])
            # additive causal mask: 0 on/below the diagonal, -400 above;
            # applied to raw scores in PSUM BEFORE exp (exp then yields ~0),
            # keeping the post-exp path free of a third-engine hop
            tn = cst_pool.tile([128, 128], FP32, tag="tn")
            nc.gpsimd.memset(tn[:], 0.0)
            nc.gpsimd.affine_select(
                out=tn[:], in_=tn[:], compare_op=mybir.AluOpType.is_ge,
                fill=-400.0, base=0, pattern=[[1, 128]], channel_multiplier=-1,
            )

            # ---- persistent SBUF tensors ----
            kt_tiles = [kt_pool.tile([128, S], BF16, tag="kt", name=f"kt{i}")
                        for i in range(4)]
            v_tiles = [v_pool.tile([128, 8 * 65], BF16, tag="v", name=f"v{i}")
                       for i in range(NKC)]
            for kc in range(NKC):
                ones_ap = v_tiles[kc][:].rearrange(
                    "p (h e) -> p h e", e=65)[:, :, 64:65]
                nc.gpsimd.memset(ones_ap, 1.0)

            # ---- resident weight tiles (all four projections) ----
            def load_w(dst, src, n=NDC):
                for dc in range(n):
                    nc.sync.dma_start(out=dst[dc][:],
                                      in_=src[dc * DC:(dc + 1) * DC, :])
            wk_tiles = [wb_pool.tile([128, GD], BF16, tag="wb", name=f"wkt{i}")
                        for i in range(NDC)]
            wv_tiles = [wb_pool.tile([128, GD], BF16, tag="wb", name=f"wvt{i}")
                        for i in range(NDC)]
            wq_tiles = [wb_pool.tile([128, GD], BF16, tag="wb", name=f"wqt{i}")
                        for i in range(NDC)]
            # issue order matters at startup: the bootstrap runs K, Q, V
            # quarters in that order; interleave Wk tiles with x tiles so
            # the first K-quarter matmul is gated by just two DMAs.
            xts_boot = []
            for dc in range(NDC):
                nc.sync.dma_start(out=wk_tiles[dc][:],
                                  in_=wk[dc * DC:(dc + 1) * DC, :])
                xt = xt_pool.tile([128, W], BF16, tag="xt")
                # x tiles from the (startup-idle) Activation queue: doubles
                # the bootstrap DMA issue rate so the first K-quarter's
                # matmuls never starve
                nc.scalar.dma_start(out=xt[:],
                                    in_=xT[dc * DC:(dc + 1) * DC, 0:W])
                xts_boot.append(xt)
            wo_tiles = {}

            def load_wo():
                # deferred past the bootstrap: these gpsimd-queue DMAs
                # otherwise delay the pool config that gates the first
                # matmul, and Wo is not needed until window 0's O-proj
                for hc in range(4):
                    for dcol in range(2):
                        t = wo_pool.tile([128, 512], FP32R, tag="wo",
                                         name=f"wot{hc}_{dcol}")
                        nc.gpsimd.dma_start(
                            out=t[:], in_=wo[hc * 128:(hc + 1) * 128,
                                             dcol * 512:(dcol + 1) * 512])
                        wo_tiles[(hc, dcol)] = t

            # ---- per-window projection quarters ----
            def load_xts(w):
                xts = []
                for dc in range(NDC):
                    xt = xt_pool.tile([128, W], BF16, tag="xt")
                    nc.sync.dma_start(
                        out=xt[:],
                        in_=xT[dc * DC:(dc + 1) * DC, w * W:(w + 1) * W])
                    xts.append(xt)
                return xts

            qt_by_w = {}

            def emit_quarter(kind, half, w, xts):
                ps = ps2.tile([128, 1024], FP32, tag="ps",
                              name=f"{kind}{half}_{w}")
                for dc in range(NDC):
                    if kind == "K":
                        for i in range(2):
                            hp = 2 * half + i
                            nc.tensor.matmul(
                                ps[:, i * 512:(i + 1) * 512],
                                wk_tiles[dc][:, hp * 128:(hp + 1) * 128],
                                xts[dc][:], start=(dc == 0),
                                stop=(dc == NDC - 1))
                    elif kind == "Q":
                        for i in range(2):
                            hp = 2 * half + i
                            nc.tensor.matmul(
                                ps[:, i * 512:(i + 1) * 512],
                                wq_tiles[dc][:, hp * 128:(hp + 1) * 128],
                                xts[dc][:], start=(dc == 0),
                                stop=(dc == NDC - 1))
                    else:  # V: [seq, hd-group] layout
                        for i in range(2):
                            sc = 2 * half + i
                            nc.tensor.matmul(
                                ps[:, i * 512:(i + 1) * 512],
                                xts[dc][:, sc * 128:(sc + 1) * 128],
                                wv_tiles[dc][:], start=(dc == 0),
                                stop=(dc == NDC - 1))
                # bias-add evictions on ACT (Identity + per-partition bias
                # AP): keeps DVE, the busier engine, off this path
                if kind == "K":
                    for i in range(2):
                        hp = 2 * half + i
                        nc.scalar.activation(
                            kt_tiles[hp][:, w * W:(w + 1) * W],
                            ps[:, i * 512:(i + 1) * 512], IDENT,
                            bias=bk_t[:, hp:hp + 1], scale=1.0)
                elif kind == "Q":
                    for i in range(2):
                        hp = 2 * half + i
                        qt = qt_pool.tile([128, W], BF16, tag="qt",
                                          name=f"qt{w}_{hp}")
                        nc.scalar.activation(
                            qt[:], ps[:, i * 512:(i + 1) * 512], IDENT,
                            bias=bq_t[:, hp:hp + 1], scale=1.0)
                        qt_by_w.setdefault(w, {})[hp] = qt
                else:
                    for i in range(2):
                        sc = 2 * half + i
                        dst = v_tiles[w * 4 + sc][:].rearrange(
                            "p (h e) -> p h e", e=65)[:, :, 0:64]
                        src = ps[:, i * 512:(i + 1) * 512].rearrange(
                            "p (h e) -> p h e", e=64)
                        nc.vector.tensor_copy(dst, src)

            # ---- O-projection for one 128-query block (interleaved) ----
            def emit_oproj(w, ao_tls, qs, tail=False):
                op2 = ps2.tile([128, 1024], FP32, tag="ps", name="op2")
                for dcol in range(2):
                    for hc in range(4):
                        nc.tensor.matmul(
                            op2[:, dcol * 512:dcol * 512 + 512],
                            ao_tls[hc][:, qs * 128:(qs + 1) * 128],
                            wo_tiles[(hc, dcol)][:],
                            start=(hc == 0), stop=(hc == 3))
                ot = out_pool.tile([128, 1024], FP32, tag="ob")
                # at the kernel tail, fan the last evictions/stores across
                # two engines/queues so they drain in parallel
                if tail and qs % 2:
                    nc.vector.tensor_copy(ot[:], op2[:])
                    nc.gpsimd.dma_start(
                        out=out[w * W + qs * 128:w * W + (qs + 1) * 128, :],
                        in_=ot[:])
                else:
                    nc.scalar.copy(ot[:], op2[:])
                    nc.sync.dma_start(
                        out=out[w * W + qs * 128:w * W + (qs + 1) * 128, :],
                        in_=ot[:])

            # ---- bootstrap: window 0's projections up front ----
            xts_cur = xts_boot
            load_w(wq_tiles, wq)
            load_w(wv_tiles, wv)
            # K quarters first: their weights+x arrive first; Wq/Wv DMAs
            # land while the K matmuls stream
            for kind in ("K", "Q", "V"):
                for half in (0, 1):
                    emit_quarter(kind, half, 0, xts_cur)
            load_wo()

            prev_ao = None
            fexp_ctr = [0]

            # ---- attention + interleaved next-window projections ----
            for w in range(NW):
                qt_tiles = qt_by_w[w]
                nkc = 4 * (w + 1)
                ao_tiles = [ao_pool.tile([128, W], FP32R, tag="ao",
                                         name=f"ao{w}_{i}") for i in range(4)]
                if w + 1 < NW:
                    xts_next = load_xts(w + 1)
                    fillers = {
                        0: [("K", 0)],
                        1: [("Q", 0), ("K", 1)],
                        2: [("Q", 1), ("V", 0)],
                        3: [("V", 1)],
                    }
                else:
                    xts_next, fillers = None, {0: [], 1: [], 2: [], 3: []}

                def emit_S(hp, kcs):
                    ets = {}
                    for kc in kcs:
                        j = kc - 4 * w
                        lo = max(j, 0) * 128
                        s2 = ps2.tile([128, 1024], FP32, tag="ps", name="s2")
                        et = exp_pool.tile([128, 1024], BF16, tag="et")
                        for par in range(2):
                            nc.tensor.matmul(
                                s2[:, par * 512 + lo:par * 512 + 512],
                                kt_tiles[hp][par * 64:(par + 1) * 64,
                                             kc * KCH:(kc + 1) * KCH],
                                qt_tiles[hp][par * 64:(par + 1) * 64, lo:W],
                                start=True, stop=True)
                        if j >= 0:
                            # additive causal mask on the diagonal block,
                            # both head-parities in one strided op
                            sd = s2[:].rearrange(
                                "p (two n) -> p two n",
                                two=2)[:, :, lo:lo + 128]
                            tn_b = tn[:].unsqueeze(1).broadcast_to(
                                [128, 2, 128])
                            nc.vector.tensor_add(sd, sd, tn_b)
                        if lo == 0:
                            # full-width chunk: alternate engines 2:1 ACT:DVE
                            fexp_ctr[0] += 1
                            if fexp_ctr[0] % 3 == 0:
                                nc.vector.tensor_scalar(
                                    et[:].bitcast(INT16), s2[:],
                                    FEXP_A * SCALE, FEXP_B,
                                    mybir.AluOpType.mult,
                                    mybir.AluOpType.add)
                            else:
                                nc.scalar.activation(et[:], s2[:], EXP,
                                                     bias=0.0, scale=SCALE)
                        else:
                            sv = s2[:].rearrange("p (two n) -> p two n",
                                                 two=2)[:, :, lo:512]
                            ev = et[:].rearrange("p (two n) -> p two n",
                                                 two=2)[:, :, lo:512]
                            nc.scalar.activation(ev, sv, EXP,
                                                 bias=0.0, scale=SCALE)
                        ets[kc] = et
                    return ets

                def emit_V(hp, o2, kcs, ets):
                    for kc in kcs:
                        j = kc - 4 * w
                        lo = max(j, 0) * 128
                        for par in range(2):
                            h = 2 * hp + par
                            nc.tensor.matmul(
                                o2[0:65, par * 512 + lo:par * 512 + 512],
                                v_tiles[kc][:, h * 65:(h + 1) * 65],
                                ets[kc][:, par * 512 + lo:par * 512 + 512],
                                start=(kc == 0), stop=(kc == nkc - 1))

                def emit_norm(hp, o2):
                    # exact 1/Z on DVE, cheap: 32x32 stream-transpose of the
                    # Z row block -> reciprocal on the [32,32] strided view
                    # (32 elems/lane instead of 1024) -> transpose back.
                    zt2 = zn_pool.tile([32, 1024], FP32, tag="zt2")
                    nc.vector.transpose(zt2[:], o2[64:96, :])
                    zs = zt2[:].rearrange("p (b c) -> p b c", c=32)[:, :, 0:1]
                    nc.vector.reciprocal(zs, zs)
                    zt3 = zn_pool.tile([32, 1024], FP32, tag="zt3")
                    nc.vector.transpose(zt3[:], zt2[:])
                    zb = zz_pool.tile([64, 1024], FP32, tag="zb")
                    nc.gpsimd.partition_broadcast(zb[:], zt3[0:1, :])
                    for par in range(2):
                        nc.vector.tensor_mul(
                            ao_tiles[hp][par * 64:(par + 1) * 64, :],
                            o2[0:64, par * 512:par * 512 + 512],
                            zb[:, par * 512:par * 512 + 512].bitcast(FP32R))

                def emit_fillers(point):
                    if prev_ao is not None:
                        emit_oproj(w - 1, prev_ao, point)
                    for kind, half in fillers[point]:
                        emit_quarter(kind, half, w + 1, xts_next)

                o2s = {}
                pending = None
                for hp in range(4):
                    o2s[hp] = ps2.tile([128, 1024], FP32, tag="ps",
                                       name=f"o2_{w}_{hp}")
                    for kc0 in range(0, nkc, 3):
                        kcs = list(range(kc0, min(kc0 + 3, nkc)))
                        ets = emit_S(hp, kcs)
                        if pending is not None:
                            p_hp, p_kcs, p_ets = pending
                            if p_hp != hp:
                                # filler matmuls BEFORE the flush of the
                                # previous hp's last (diagonal) batch: the
                                # PE streams oproj/projection work while
                                # that batch's exp chain drains
                                emit_fillers(p_hp)
                            emit_V(p_hp, o2s[p_hp], p_kcs, p_ets)
                            if p_hp != hp:
                                emit_norm(p_hp, o2s[p_hp])
                        pending = (hp, kcs, ets)
                p_hp, p_kcs, p_ets = pending
                emit_fillers(3)
                emit_V(p_hp, o2s[p_hp], p_kcs, p_ets)
                emit_norm(p_hp, o2s[p_hp])
                prev_ao = ao_tiles

            # ---- output projection for the final window ----
            for qs in range(4):
                emit_oproj(NW - 1, prev_ao, qs, tail=True)

    nc.compile()
    return nc


def _get_program():
    if "nc" not in _CACHE:
        _CACHE["nc"] = _build_program()
    return _CACHE["nc"]


def _install_ntff_hook():
    """The agent image's antenv lacks axon_hooks; shim it and register the
    ctypes NTFF profiling hook so trace=True yields exec_time_ns."""
    import sys, types
    if "antenv.axon_hooks" in sys.modules:
        return
    try:
        import antenv
        mod = types.ModuleType("antenv.axon_hooks")
        _h = [None]
        mod.set_axon_ntff_profile_hook = lambda h: _h.__setitem__(0, h)
        mod.get_axon_ntff_profile_hook = lambda: _h[0]
        sys.modules["antenv.axon_hooks"] = mod
        antenv.axon_hooks = mod
        from trn_agent_boot.trn_boot import _ntff_profile_via_ctypes
        mod.set_axon_ntff_profile_hook(
            _ntff_profile_via_ctypes("/opt/axon/libaxon_pjrt.so"))
    except Exception as e:  # degrade: run without tracing
        print(f"NTFF hook install failed ({e}); tracing disabled")


def _run(inputs, trace=False):
    x = np.asarray(inputs["x"], dtype=np.float32)
    Wq = np.asarray(inputs["Wq"], dtype=np.float32)
    Wk = np.asarray(inputs["Wk"], dtype=np.float32)
    Wv = np.asarray(inputs["Wv"], dtype=np.float32)
    Wo = np.asarray(inputs["Wo"], dtype=np.float32)
    bq = np.asarray(inputs["bq"], dtype=np.float32)
    bk = np.asarray(inputs["bk"], dtype=np.float32)
    bv = np.asarray(inputs["bv"], dtype=np.float32)
    bo = np.asarray(inputs["bo"], dtype=np.float32)

    if trace:
        _install_ntff_hook()
    nc = _get_program()
    bf = ml_dtypes.bfloat16
    in_maps = []
    for c in range(NCORES):
        b, g = divmod(c, 2)
        sl = slice(g * GD, (g + 1) * GD)
        in_maps.append({
            "xT": np.ascontiguousarray(x[b].T).astype(bf),
            "wq": np.ascontiguousarray(Wq[:, sl]).astype(bf),
            "wk": np.ascontiguousarray(Wk[:, sl]).astype(bf),
            "wv": np.ascontiguousarray(Wv[:, sl]).astype(bf),
            "wo": np.ascontiguousarray(Wo[sl, :]),
            "bq2": np.ascontiguousarray(bq[sl].reshape(4, 128).T),
            "bk2": np.ascontiguousarray(bk[sl].reshape(4, 128).T),
        })
    res = run_bass_kernel_spmd(nc, in_maps, list(range(NCORES)), trace=trace)
    outp = np.empty((B, S, D), dtype=np.float32)
    # bv correction: attention rows sum to 1, so x @ Wv + bv contributes
    # attn@V + bv per row; bv flows through Wo as a constant row vector.
    corr = (bv @ Wo + bo).astype(np.float32)
    for b in range(B):
        outp[b] = res.results[2 * b]["out"] + res.results[2 * b + 1]["out"] + corr
    return outp, res


def kernel(**inputs):
    outp, _ = _run(inputs, trace=False)
    return outp


def kernel_traced(**inputs):
    outp, res = _run(inputs, trace=True)
    return outp, res


# revision 48
# speedup vs baseline: 1.0083x; 1.0083x over previous
"""Causal multi-head attention (B=4, S=2048, D=1024, H=16, hd=64) on 8
Trainium2 NeuronCores.

Sharding: batch (4-way) x head-group (2-way). Core c handles batch c//2 and
heads [8*(c%2), 8*(c%2)+8). Each core computes its heads' contribution to the
output projection; the host sums the two partials per batch and adds bo.

Per-core device program:
  - Projections run in bf16 (x and Wq/Wk/Wv pre-cast on the host): the
    1024-deep contraction washes the quantization out, FWL doubles weight
    loads, and x DMA traffic halves. Q/K/V results are evicted to fp32r
    (K/Q) and bf16 (V) for the attention matmuls.
  - K/V/Q projections are emitted per 512-query window as six "quarters"
    (K/V/Q x two head-pairs, one [128,1024] PSUM tile each), interleaved
    into the PREVIOUS window's attention stream as PE filler so the PE
    never idles long enough for HAM to re-throttle the clock.
  - Attention per window in the transposed layout: scoresT = K_chunk @ Q^T
    (row-tiled K=64 matmuls, head parities at partitions 0-63/64-127 run
    concurrently), exp(s/8) split across engines: most chunks on ACT
    (spline exp), every third full-width chunk on DVE via the Schraudolph
    bit-trick (int16(A*s + B) reinterpreted as bf16; ~3% rel err, washes
    out after softmax normalization). Causal block-skipping plus a
    triangular bf16 mask on diagonal subblocks. attn-out accumulates via
    bf16 V_aug matmuls whose fused ones-column also yields the softmax
    denominator Z (PSUM row 64).
  - Normalization: 1/Z computed EXACTLY on DVE with zero ACT table
    pressure: 32x32 stream-transpose of the Z row block, reciprocal on the
    [32,32]-strided column view (8 cyc/elem on only 32 elems/lane instead
    of 1024), transpose back, gpsimd partition_broadcast, multiply.
  - Output projection of window w-1 interleaved into window w (more PE
    filler); eviction via ACT copy; partial [S, D] DMA'd out.
"""
import numpy as np
import ml_dtypes

import concourse.mybir as mybir
from concourse import bacc
from concourse.tile import TileContext
from concourse.bass_utils import run_bass_kernel_spmd

FP32 = mybir.dt.float32
FP32R = mybir.dt.float32r
BF16 = mybir.dt.bfloat16
INT16 = mybir.dt.int16
EXP = mybir.ActivationFunctionType.Exp
IDENT = mybir.ActivationFunctionType.Identity

B, S, D = 4, 2048, 1024
H, HD = 16, 64
NCORES = 8
HPG = 8              # heads per group (per core)
GD = HPG * HD        # 512: group head-dim width
W = 512              # query window
NW = S // W          # 4
KCH = 128            # key chunk
NKC = S // KCH       # 16
DC = 128             # D contraction chunk
NDC = D // DC        # 8
SCALE = 1.0 / 8.0    # 1/sqrt(hd)

# Schraudolph fast-exp, bf16 flavor: exp(y) ~= bits(int16(A*y + B)) as bf16.
# C=5.5 calibrated for min max-rel-err (~3.3%) over y in [-8, 4].
FEXP_A = 128.0 / float(np.log(2.0))
FEXP_B = 127.0 * 128.0 - 5.5

_CACHE = {}


def _build_program():
    nc = bacc.Bacc("TRN2", target_bir_lowering=False, debug=False,
                   num_devices=NCORES)

    xT = nc.dram_tensor("xT", [D, S], BF16, kind="ExternalInput").ap()
    wq = nc.dram_tensor("wq", [D, GD], BF16, kind="ExternalInput").ap()
    wk = nc.dram_tensor("wk", [D, GD], BF16, kind="ExternalInput").ap()
    wv = nc.dram_tensor("wv", [D, GD], BF16, kind="ExternalInput").ap()
    wo = nc.dram_tensor("wo", [GD, D], FP32, kind="ExternalInput").ap()
    bq2 = nc.dram_tensor("bq2", [128, 4], FP32, kind="ExternalInput").ap()
    bk2 = nc.dram_tensor("bk2", [128, 4], FP32, kind="ExternalInput").ap()
    out = nc.dram_tensor("out", [S, D], FP32, kind="ExternalOutput").ap()

    with TileContext(nc) as tc:
        with (
            tc.tile_pool(name="xt", bufs=10) as xt_pool,
            tc.tile_pool(name="wb", bufs=24) as wb_pool,
            tc.tile_pool(name="wo", bufs=8) as wo_pool,
            tc.tile_pool(name="kt", bufs=4) as kt_pool,
            tc.tile_pool(name="vst", bufs=16) as v_pool,
            tc.tile_pool(name="qt", bufs=8) as qt_pool,
            tc.tile_pool(name="et", bufs=8) as exp_pool,
            tc.tile_pool(name="ao", bufs=8) as ao_pool,
            tc.tile_pool(name="zz", bufs=2) as zz_pool,
            tc.tile_pool(name="zn", bufs=1) as zn_pool,
            tc.tile_pool(name="cst", bufs=1) as cst_pool,
            tc.tile_pool(name="ob", bufs=3) as out_pool,
            tc.tile_pool(name="ps", bufs=4, space="PSUM") as ps2,
        ):
            # ---- constants: biases, triangular mask ----
            bq_t = cst_pool.tile([128, 4], FP32, tag="bq")
            bk_t = cst_pool.tile([128, 4], FP32, tag="bk")
            nc.sync.dma_start(out=bq_t[:], in_=bq2[:])
            nc.sync.dma_start(out=bk_t[:], in_=bk2[:])
            # additive causal mask: 0 on/below the diagonal, -400 above;
            # applied to raw scores in PSUM BEFORE exp (exp then yields ~0),
            # keeping the post-exp path free of a third-engine hop
            tn = cst_pool.tile([128, 128], FP32, tag="tn")
            nc.gpsimd.memset(tn[:], 0.0)
            nc.gpsimd.affine_select(
                out=tn[:], in_=tn[:], compare_op=mybir.AluOpType.is_ge,
                fill=-400.0, base=0, pattern=[[1, 128]], channel_multiplier=-1,
            )

            # ---- persistent SBUF tensors ----
            kt_tiles = [kt_pool.tile([128, S], BF16, tag="kt", name=f"kt{i}")
                        for i in range(4)]
            v_tiles = [v_pool.tile([128, 8 * 65], BF16, tag="v", name=f"v{i}")
                       for i in range(NKC)]
            for kc in range(NKC):
                ones_ap = v_tiles[kc][:].rearrange(
                    "p (h e) -> p h e", e=65)[:, :, 64:65]
                nc.gpsimd.memset(ones_ap, 1.0)

            # ---- resident weight tiles (all four projections) ----
            def load_w(dst, src, n=NDC):
                for dc in range(n):
                    nc.sync.dma_start(out=dst[dc][:],
                                      in_=src[dc * DC:(dc + 1) * DC, :])
            wk_tiles = [wb_pool.tile([128, GD], BF16, tag="wb", name=f"wkt{i}")
                        for i in range(NDC)]
            wv_tiles = [wb_pool.tile([128, GD], BF16, tag="wb", name=f"wvt{i}")
                        for i in range(NDC)]
            wq_tiles = [wb_pool.tile([128, GD], BF16, tag="wb", name=f"wqt{i}")
                        for i in range(NDC)]
            # issue order matters at startup: the bootstrap runs K, Q, V
            # quarters in that order; interleave Wk tiles with x tiles so
            # the first K-quarter matmul is gated by just two DMAs.
            xts_boot = []
            for dc in range(NDC):
                nc.sync.dma_start(out=wk_tiles[dc][:],
                                  in_=wk[dc * DC:(dc + 1) * DC, :])
                xt = xt_pool.tile([128, W], BF16, tag="xt")
                # x tiles from the (startup-idle) Activation queue: doubles
                # the bootstrap DMA issue rate so the first K-quarter's
                # matmuls never starve
                nc.scalar.dma_start(out=xt[:],
                                    in_=xT[dc * DC:(dc + 1) * DC, 0:W])
                xts_boot.append(xt)
            wo_tiles = {}

            def load_wo():
                # deferred past the bootstrap: these gpsimd-queue DMAs
                # otherwise delay the pool config that gates the first
                # matmul, and Wo is not needed until window 0's O-proj
                for hc in range(4):
                    for dcol in range(2):
                        t = wo_pool.tile([128, 512], FP32R, tag="wo",
                                         name=f"wot{hc}_{dcol}")
                        nc.gpsimd.dma_start(
                            out=t[:], in_=wo[hc * 128:(hc + 1) * 128,
                                             dcol * 512:(dcol + 1) * 512])
                        wo_tiles[(hc, dcol)] = t

            # ---- per-window projection quarters ----
            def load_xts(w):
                xts = []
                for dc in range(NDC):
                    xt = xt_pool.tile([128, W], BF16, tag="xt")
                    nc.sync.dma_start(
                        out=xt[:],
                        in_=xT[dc * DC:(dc + 1) * DC, w * W:(w + 1) * W])
                    xts.append(xt)
                return xts

            qt_by_w = {}

            def emit_quarter(kind, half, w, xts):
                ps = ps2.tile([128, 1024], FP32, tag="ps",
                              name=f"{kind}{half}_{w}")
                for dc in range(NDC):
                    if kind == "K":
                        for i in range(2):
                            hp = 2 * half + i
                            nc.tensor.matmul(
                                ps[:, i * 512:(i + 1) * 512],
                                wk_tiles[dc][:, hp * 128:(hp + 1) * 128],
                                xts[dc][:], start=(dc == 0),
                                stop=(dc == NDC - 1))
                    elif kind == "Q":
                        for i in range(2):
                            hp = 2 * half + i
                            nc.tensor.matmul(
                                ps[:, i * 512:(i + 1) * 512],
                                wq_tiles[dc][:, hp * 128:(hp + 1) * 128],
                                xts[dc][:], start=(dc == 0),
                                stop=(dc == NDC - 1))
                    else:  # V: [seq, hd-group] layout
                        for i in range(2):
                            sc = 2 * half + i
                            nc.tensor.matmul(
                                ps[:, i * 512:(i + 1) * 512],
                                xts[dc][:, sc * 128:(sc + 1) * 128],
                                wv_tiles[dc][:], start=(dc == 0),
                                stop=(dc == NDC - 1))
                # bias-add evictions on ACT (Identity + per-partition bias
                # AP): keeps DVE, the busier engine, off this path
                if kind == "K":
                    for i in range(2):
                        hp = 2 * half + i
                        nc.scalar.activation(
                            kt_tiles[hp][:, w * W:(w + 1) * W],
                            ps[:, i * 512:(i + 1) * 512], IDENT,
                            bias=bk_t[:, hp:hp + 1], scale=1.0)
                elif kind == "Q":
                    for i in range(2):
                        hp = 2 * half + i
                        qt = qt_pool.tile([128, W], BF16, tag="qt",
                                          name=f"qt{w}_{hp}")
                        nc.scalar.activation(
                            qt[:], ps[:, i * 512:(i + 1) * 512], IDENT,
                            bias=bq_t[:, hp:hp + 1], scale=1.0)
                        qt_by_w.setdefault(w, {})[hp] = qt
                else:
                    for i in range(2):
                        sc = 2 * half + i
                        dst = v_tiles[w * 4 + sc][:].rearrange(
                            "p (h e) -> p h e", e=65)[:, :, 0:64]
                        src = ps[:, i * 512:(i + 1) * 512].rearrange(
                            "p (h e) -> p h e", e=64)
                        nc.vector.tensor_copy(dst, src)

            # ---- O-projection for one 128-query block (interleaved) ----
            def emit_oproj(w, ao_tls, qs, tail=False):
                op2 = ps2.tile([128, 1024], FP32, tag="ps", name="op2")
                for dcol in range(2):
                    for hc in range(4):
                        nc.tensor.matmul(
                            op2[:, dcol * 512:dcol * 512 + 512],
                            ao_tls[hc][:, qs * 128:(qs + 1) * 128],
                            wo_tiles[(hc, dcol)][:],
                            start=(hc == 0), stop=(hc == 3))
                ot = out_pool.tile([128, 1024], FP32, tag="ob")
                # at the kernel tail, fan the last evictions/stores across
                # two engines/queues so they drain in parallel
                if tail and qs % 2:
                    nc.vector.tensor_copy(ot[:], op2[:])
                    nc.gpsimd.dma_start(
                        out=out[w * W + qs * 128:w * W + (qs + 1) * 128, :],
                        in_=ot[:])
                else:
                    nc.scalar.copy(ot[:], op2[:])
                    nc.sync.dma_start(
                        out=out[w * W + qs * 128:w * W + (qs + 1) * 128, :],
                        in_=ot[:])

            # ---- bootstrap: window 0's projections up front ----
            xts_cur = xts_boot
            load_w(wq_tiles, wq)
            load_w(wv_tiles, wv)
            # K quarters first: their weights+x arrive first; Wq/Wv DMAs
            # land while the K matmuls stream
            for kind in ("K", "Q", "V"):
                for half in (0, 1):
                    emit_quarter(kind, half, 0, xts_cur)
            load_wo()

            prev_ao = None
            fexp_ctr = [0]

            # ---- attention + interleaved next-window projections ----
            for w in range(NW):
                qt_tiles = qt_by_w[w]
                nkc = 4 * (w + 1)
                ao_tiles = [ao_pool.tile([128, W], FP32R, tag="ao",
                                         name=f"ao{w}_{i}") for i in range(4)]
                if w + 1 < NW:
                    xts_next = load_xts(w + 1)
                    fillers = {
                        0: [("K", 0)],
                        1: [("Q", 0), ("K", 1)],
                        2: [("Q", 1), ("V", 0)],
                        3: [("V", 1)],
                    }
                else:
                    xts_next, fillers = None, {0: [], 1: [], 2: [], 3: []}

                def emit_S(hp, kcs):
                    ets = {}
                    for kc in kcs:
                        j = kc - 4 * w
                        lo = max(j, 0) * 128
                        s2 = ps2.tile([128, 1024], FP32, tag="ps", name="s2")
                        et = exp_pool.tile([128, 1024], BF16, tag="et")
                        for par in range(2):
                            nc.tensor.matmul(
                                s2[:, par * 512 + lo:par * 512 + 512],
                                kt_tiles[hp][par * 64:(par + 1) * 64,
                                             kc * KCH:(kc + 1) * KCH],
                                qt_tiles[hp][par * 64:(par + 1) * 64, lo:W],
                                start=True, stop=True)
                        if j >= 0:
                            # additive causal mask on the diagonal block,
                            # both head-parities in one strided op
                            sd = s2[:].rearrange(
                                "p (two n) -> p two n",
                                two=2)[:, :, lo:lo + 128]
                            tn_b = tn[:].unsqueeze(1).broadcast_to(
                                [128, 2, 128])
                            nc.vector.tensor_add(sd, sd, tn_b)
                        if lo == 0:
                            # full-width chunk: alternate engines 2:1 ACT:DVE
                            fexp_ctr[0] += 1
                            if fexp_ctr[0] % 3 == 0:
                                nc.vector.tensor_scalar(
                                    et[:].bitcast(INT16), s2[:],
                                    FEXP_A * SCALE, FEXP_B,
                                    mybir.AluOpType.mult,
                                    mybir.AluOpType.add)
                            else:
                                nc.scalar.activation(et[:], s2[:], EXP,
                                                     bias=0.0, scale=SCALE)
                        else:
                            sv = s2[:].rearrange("p (two n) -> p two n",
                                                 two=2)[:, :, lo:512]
                            ev = et[:].rearrange("p (two n) -> p two n",
                                                 two=2)[:, :, lo:512]
                            nc.scalar.activation(ev, sv, EXP,
                                                 bias=0.0, scale=SCALE)
                        ets[kc] = et
                    return ets

                def emit_V(hp, o2, kcs, ets):
                    for kc in kcs:
                        j = kc - 4 * w
                        lo = max(j, 0) * 128
                        for par in range(2):
                            h = 2 * hp + par
                            nc.tensor.matmul(
                                o2[0:65, par * 512 + lo:par * 512 + 512],
                                v_tiles[kc][:, h * 65:(h + 1) * 65],
                                ets[kc][:, par * 512 + lo:par * 512 + 512],
                                start=(kc == 0), stop=(kc == nkc - 1))

                def emit_norm(hp, o2):
                    # exact 1/Z on DVE, cheap: 32x32 stream-transpose of the
                    # Z row block -> reciprocal on the [32,32] strided view
                    # (32 elems/lane instead of 1024) -> transpose back.
                    zt2 = zn_pool.tile([32, 1024], FP32, tag="zt2")
                    nc.vector.transpose(zt2[:], o2[64:96, :])
                    zs = zt2[:].rearrange("p (b c) -> p b c", c=32)[:, :, 0:1]
                    nc.vector.reciprocal(zs, zs)
                    zt3 = zn_pool.tile([32, 1024], FP32, tag="zt3")
                    nc.vector.transpose(zt3[:], zt2[:])
                    zb = zz_pool.tile([64, 1024], FP32, tag="zb")
                    nc.gpsimd.partition_broadcast(zb[:], zt3[0:1, :])
                    for par in range(2):
                        nc.vector.tensor_mul(
                            ao_tiles[hp][par * 64:(par + 1) * 64, :],
                            o2[0:64, par * 512:par * 512 + 512],
                            zb[:, par * 512:par * 512 + 512].bitcast(FP32R))

                def emit_fillers(point):
                    if prev_ao is not None:
                        emit_oproj(w - 1, prev_ao, point)
                    for kind, half in fillers[point]:
                        emit_quarter(kind, half, w + 1, xts_next)

                o2s = {}
                pending = None
                for hp in range(4):
                    o2s[hp] = ps2.tile([128, 1024], FP32, tag="ps",
                                       name=f"o2_{w}_{hp}")
                    for kc0 in range(0, nkc, 3):
                        kcs = list(range(kc0, min(kc0 + 3, nkc)))
                        ets = emit_S(hp, kcs)
                        if pending is not None:
                            p_hp, p_kcs, p_ets = pending
                            if p_hp != hp:
                                # filler matmuls BEFORE the flush of the
                                # previous hp's last (diagonal) batch: the
                                # PE streams oproj/projection work while
                                # that batch's exp chain drains
                                emit_fillers(p_hp)
                            emit_V(p_hp, o2s[p_hp], p_kcs, p_ets)
                            if p_hp != hp:
                                emit_norm(p_hp, o2s[p_hp])
                        pending = (hp, kcs, ets)
                p_hp, p_kcs, p_ets = pending
                if w + 1 < NW:
                    emit_fillers(3)
                    emit_V(p_hp, o2s[p_hp], p_kcs, p_ets)
                    emit_norm(p_hp, o2s[p_hp])
                else:
                    # last window: no next-window S work follows, so keep
                    # the final filler for AFTER the norm emission — the PE
                    # streams it while the norm chain drains, instead of
                    # idling before the tail O-projection
                    emit_V(p_hp, o2s[p_hp], p_kcs, p_ets)
                    emit_norm(p_hp, o2s[p_hp])
                    emit_fillers(3)
                prev_ao = ao_tiles

            # ---- output projection for the final window ----
            for qs in range(4):
                emit_oproj(NW - 1, prev_ao, qs, tail=True)

    nc.compile()
    return nc


def _get_program():
    if "nc" not in _CACHE:
        _CACHE["nc"] = _build_program()
    return _CACHE["nc"]


def _install_ntff_hook():
    """The agent image's antenv lacks axon_hooks; shim it and register the
    ctypes NTFF profiling hook so trace=True yields exec_time_ns."""
    import sys, types
    if "antenv.axon_hooks" in sys.modules:
        return
    try:
        import antenv
        mod = types.ModuleType("antenv.axon_hooks")
        _h = [None]
        mod.set_axon_ntff_profile_hook = lambda h: _h.__setitem__(0, h)
        mod.get_axon_ntff_profile_hook = lambda: _h[0]
        sys.modules["antenv.axon_hooks"] = mod
        antenv.axon_hooks = mod
        from trn_agent_boot.trn_boot import _ntff_profile_via_ctypes
        mod.set_axon_ntff_profile_hook(
            _ntff_profile_via_ctypes("/opt/axon/libaxon_pjrt.so"))
    except Exception as e:  # degrade: run without tracing
        print(f"NTFF hook install failed ({e}); tracing disabled")


def _run(inputs, trace=False):
    x = np.asarray(inputs["x"], dtype=np.float32)
    Wq = np.asarray(inputs["Wq"], dtype=np.float32)
    Wk = np.asarray(inputs["Wk"], dtype=np.float32)
    Wv = np.asarray(inputs["Wv"], dtype=np.float32)
    Wo = np.asarray(inputs["Wo"], dtype=np.float32)
    bq = np.asarray(inputs["bq"], dtype=np.float32)
    bk = np.asarray(inputs["bk"], dtype=np.float32)
    bv = np.asarray(inputs["bv"], dtype=np.float32)
    bo = np.asarray(inputs["bo"], dtype=np.float32)

    if trace:
        _install_ntff_hook()
    nc = _get_program()
    bf = ml_dtypes.bfloat16
    in_maps = []
    for c in range(NCORES):
        b, g = divmod(c, 2)
        sl = slice(g * GD, (g + 1) * GD)
        in_maps.append({
            "xT": np.ascontiguousarray(x[b].T).astype(bf),
            "wq": np.ascontiguousarray(Wq[:, sl]).astype(bf),
            "wk": np.ascontiguousarray(Wk[:, sl]).astype(bf),
            "wv": np.ascontiguousarray(Wv[:, sl]).astype(bf),
            "wo": np.ascontiguousarray(Wo[sl, :]),
            "bq2": np.ascontiguousarray(bq[sl].reshape(4, 128).T),
            "bk2": np.ascontiguousarray(bk[sl].reshape(4, 128).T),
        })
    res = run_bass_kernel_spmd(nc, in_maps, list(range(NCORES)), trace=trace)
    outp = np.empty((B, S, D), dtype=np.float32)
    # bv correction: attention rows sum to 1, so x @ Wv + bv contributes
    # attn@V + bv per row; bv flows through Wo as a constant row vector.
    corr = (bv @ Wo + bo).astype(np.float32)
    for b in range(B):
        outp[b] = res.results[2 * b]["out"] + res.results[2 * b + 1]["out"] + corr
    return outp, res


def kernel(**inputs):
    outp, _ = _run(inputs, trace=False)
    return outp


def kernel_traced(**inputs):
    outp, res = _run(inputs, trace=True)
    return outp, res
